# revision 2
# baseline (speedup 1.0000x reference)
"""Trainium2 Bass kernel for nn_NodeNet (GNN message passing) — v2.

All-bf16 design (fp8 DoubleRow was measured numerically and busts the 2e-2
error gate). Data-parallel over graphs across 8 NeuronCores.

Key structural ideas vs the 420us baseline:
  * feature_enc is never materialized. Its entire contribution to the edge
    MLP's first layer is a per-graph vector z = hsum @ M with
    M = node_w3 @ edge_w1[:64] (host-precomputed), plus a per-channel
    constant bias bz = edge_w1[:64]^T nb3 + eb1 folded into the h1
    activation bias. This kills the [64, E] broadcast copy (43us of DVE in
    the baseline) and the node third layer.
  * z enters the edge L1 matmul as 4 extra contraction rows: the rhs tile is
    [attr^T (64 rows); graph-indicator (4 rows)] and the lhsT is
    [W1b (64 rows); zT tile-slice (4 rows)]. One matmul per output half.
  * Edge L3 packs two edges per output column (out [128, 256] per 512-edge
    tile, partitions = (edge parity, channel)) using col-group tile
    placement, halving both L3 PSUM evacuation and the out copy.
  * PSUM evacuation (the hard constraint: only scalar+vector reach PSUM) is
    split so scalar (153 Ge/s) and vector (123 Ge/s) finish together under
    the tensor time; hsum reduce runs in bf16 2x mode.
  * Output DMA'd as bf16 and expanded on host.
"""

import os
import sys

import ml_dtypes
import numpy as np

BF16NP = ml_dtypes.bfloat16

if "/opt/trn_rl_repo" not in sys.path and os.path.isdir("/opt/trn_rl_repo"):
    sys.path.insert(0, "/opt/trn_rl_repo")

import concourse.bacc as bacc
import concourse.tile as tile
from concourse import mybir
from concourse.bass_utils import run_bass_kernel_spmd

G, ODE, NDATA, H, EA, EPG = 4096, 64, 32, 256, 64, 128
E = G * EPG
NCORES = 8
GC = G // NCORES           # graphs per core (512)
RC = GC * NDATA            # node-MLP rows per core (16384)
EC = GC * EPG              # edges per core (65536)
TN = 512                   # tile free size
NT_N = RC // TN            # node tiles (32)
NT_E = EC // TN            # edge tiles (128)
GT = TN // NDATA           # graphs per node tile (16)
GE = TN // EPG             # graphs per edge tile (4)

F32 = mybir.dt.float32
BF16 = mybir.dt.bfloat16
RELU = mybir.ActivationFunctionType.Relu
IDENT = mybir.ActivationFunctionType.Identity
ADD = mybir.AluOpType.add
MAX = mybir.AluOpType.max
BYPASS = mybir.AluOpType.bypass
AXX = mybir.AxisListType.X

_PROGRAMS = {}
last_results = None


def _install_trace_shim():
    import types

    if "antenv.axon_hooks" in sys.modules:
        return
    try:
        mod = types.ModuleType("antenv.axon_hooks")
        mod._hook = None
        mod.set_axon_ntff_profile_hook = lambda h: setattr(mod, "_hook", h)
        mod.get_axon_ntff_profile_hook = lambda: mod._hook
        sys.modules["antenv.axon_hooks"] = mod
        import antenv

        antenv.axon_hooks = mod
        from trn_agent_boot.trn_boot import _ntff_profile_via_ctypes

        hook = _ntff_profile_via_ctypes("/opt/axon/libaxon_pjrt.so")
        if hook is not None:
            mod.set_axon_ntff_profile_hook(hook)
    except Exception:
        pass


def _build():
    nc = bacc.Bacc("TRN2", target_bir_lowering=False)
    xT_d = nc.dram_tensor("xT", [128, RC], BF16, kind="ExternalInput")
    attr_d = nc.dram_tensor("attrT2", [68, EC], BF16, kind="ExternalInput")
    wrep_d = nc.dram_tensor("wrep", [64, NT_E, 2, 128], BF16, kind="ExternalInput")
    nw1_d = nc.dram_tensor("nw1", [128, H], BF16, kind="ExternalInput")
    nw2_d = nc.dram_tensor("nw2", [128, 2, H], BF16, kind="ExternalInput")
    ew2_d = nc.dram_tensor("ew2", [128, 2, H], BF16, kind="ExternalInput")
    ew3_d = nc.dram_tensor("ew3", [128, 2, ODE], BF16, kind="ExternalInput")
    mt_d = nc.dram_tensor("Mt", [128, 2, H], BF16, kind="ExternalInput")
    nb1_d = nc.dram_tensor("nb1", [128, 2], F32, kind="ExternalInput")
    nb2_d = nc.dram_tensor("nb2", [128, 2], F32, kind="ExternalInput")
    bz_d = nc.dram_tensor("bz", [128, 2], F32, kind="ExternalInput")
    eb2_d = nc.dram_tensor("eb2", [128, 2], F32, kind="ExternalInput")
    eb3s_d = nc.dram_tensor("eb3s", [128, 1], F32, kind="ExternalInput")
    out_d = nc.dram_tensor("outP", [128, EC // 2], BF16, kind="ExternalOutput")

    with tile.TileContext(nc) as tc:
        with (
            tc.tile_pool(name="consts", bufs=1) as consts,
            tc.tile_pool(name="xin", bufs=4) as xin,
            tc.tile_pool(name="hid", bufs=3) as hid,
            tc.tile_pool(name="psmm", bufs=4, space="PSUM") as psmm,
            tc.tile_pool(name="psl2", bufs=2, space="PSUM") as psl2,
        ):
            # --- persistent SBUF ---
            w = {}
            for name, d in (("nw1", nw1_d), ("nw2", nw2_d), ("ew2", ew2_d),
                            ("ew3", ew3_d), ("Mt", mt_d), ("nb1", nb1_d),
                            ("nb2", nb2_d), ("bz", bz_d), ("eb2", eb2_d),
                            ("eb3s", eb3s_d)):
                w[name] = consts.tile(list(d.shape), d.dtype, tag=name, name=name)
                nc.sync.dma_start(w[name], d[:])
            arena = consts.tile([68, NT_E, 2, 128], BF16, tag="arena", name="arena")
            nc.sync.dma_start(arena[0:64], wrep_d[:])
            hsum = consts.tile([128, 2, GC], BF16, tag="hsum", name="hsum")

            # --- node stage ---
            for t in range(NT_N):
                xt = xin.tile([128, TN], BF16, tag="xt")
                nc.sync.dma_start(xt, xT_d[:, t * TN:(t + 1) * TN])
                ps_a = psmm.tile([128, TN], F32, tag="mm")
                ps_b = psmm.tile([128, TN], F32, tag="mm")
                nc.tensor.matmul(ps_a, w["nw1"][:, 0:128], xt, start=True, stop=True)
                nc.tensor.matmul(ps_b, w["nw1"][:, 128:256], xt, start=True, stop=True)
                h1 = hid.tile([128, 2, TN], BF16, tag="h1")
                nc.scalar.activation(h1[:, 0], ps_a, RELU, bias=w["nb1"][:, 0:1])
                nc.vector.tensor_scalar(out=h1[:, 1], in0=ps_b,
                                        scalar1=w["nb1"][:, 1:2], scalar2=0.0,
                                        op0=ADD, op1=MAX)
                ps2 = psl2.tile([128, 2, TN], F32, tag="l2")
                for ho in (0, 1):
                    for k in (0, 1):
                        nc.tensor.matmul(ps2[:, ho], w["nw2"][:, k, 128 * ho:128 * ho + 128],
                                         h1[:, k], start=(k == 0), stop=(k == 1))
                h2 = hid.tile([128, 2, TN], BF16, tag="h2")
                nc.scalar.activation(h2[:, 0], ps2[:, 0], RELU, bias=w["nb2"][:, 0:1])
                nc.scalar.activation(h2[:, 1], ps2[:, 1], RELU, bias=w["nb2"][:, 1:2])
                with nc.allow_low_precision(reason="bf16 hsum feeds bf16 matmul"):
                    nc.vector.reduce_sum(
                        out=hsum[:, :, t * GT:(t + 1) * GT],
                        in_=h2.rearrange("p k (g d) -> p (k g) d", d=NDATA),
                        axis=AXX,
                    )

            # --- z stage: zT[g, :] = hsum[:, g]^T @ M ---
            hsum_r = hsum.rearrange("p k (t j) -> p k j t", j=4)
            for j in range(4):
                psz = psmm.tile([128, 256], F32, tag="mm")
                for k in (0, 1):
                    nc.tensor.matmul(psz, hsum_r[:, k, j], w["Mt"][:, k, :],
                                     start=(k == 0), stop=(k == 1))
                zs = consts.tile([128, 256], BF16, tag=f"zs{j}", name=f"zs{j}")
                nc.scalar.copy(zs, psz)
                nc.sync.dma_start(
                    arena[64 + j:65 + j].rearrange("p t h c -> p (t h c)"),
                    zs,
                )

            # --- edge stage ---
            for t in range(NT_E):
                rt = xin.tile([68, TN], BF16, tag="rt")
                nc.sync.dma_start(rt, attr_d[:, t * TN:(t + 1) * TN])
                ps_a = psmm.tile([128, TN], F32, tag="mm")
                ps_b = psmm.tile([128, TN], F32, tag="mm")
                nc.tensor.matmul(ps_a, arena[0:68, t, 0, :], rt, start=True, stop=True)
                nc.tensor.matmul(ps_b, arena[0:68, t, 1, :], rt, start=True, stop=True)
                e1 = hid.tile([128, 2, TN], BF16, tag="h1")
                nc.scalar.activation(e1[:, 0], ps_a, RELU, bias=w["bz"][:, 0:1])
                nc.vector.tensor_scalar(out=e1[:, 1], in0=ps_b,
                                        scalar1=w["bz"][:, 1:2], scalar2=0.0,
                                        op0=ADD, op1=MAX)
                ps2 = psl2.tile([128, 2, TN], F32, tag="l2")
                for ho in (0, 1):
                    for k in (0, 1):
                        nc.tensor.matmul(ps2[:, ho], w["ew2"][:, k, 128 * ho:128 * ho + 128],
                                         e1[:, k], start=(k == 0), stop=(k == 1))
                e2 = hid.tile([128, 2, TN], BF16, tag="h2")
                nc.scalar.activation(e2[:, 0], ps2[:, 0], RELU, bias=w["eb2"][:, 0:1])
                nc.vector.tensor_scalar(out=e2[:, 1], in0=ps2[:, 1],
                                        scalar1=w["eb2"][:, 1:2], scalar2=0.0,
                                        op0=ADD, op1=MAX)
                ps3 = psmm.tile([128, 256], F32, tag="mm")
                e2r = e2.rearrange("p k (c q) -> p k q c", q=2)
                for q in (0, 1):
                    for k in (0, 1):
                        nc.tensor.matmul(ps3[64 * q:64 * q + 64, :], w["ew3"][:, k, :],
                                         e2r[:, k, q], start=(k == 0), stop=(k == 1))
                ot = hid.tile([128, 256], BF16, tag="ot")
                if t % 2 == 0:
                    nc.scalar.activation(ot, ps3, IDENT, bias=w["eb3s"])
                else:
                    nc.vector.tensor_scalar(out=ot, in0=ps3, scalar1=w["eb3s"],
                                            scalar2=0.0, op0=ADD, op1=BYPASS)
                nc.sync.dma_start(out_d[:, t * 256:(t + 1) * 256], ot)
    nc.finalize()
    return nc


def _get_program():
    if "v2" not in _PROGRAMS:
        _PROGRAMS["v2"] = _build()
    return _PROGRAMS["v2"]


def _f32(a):
    return np.asarray(a, dtype=np.float32)


def _host_arrays(kw):
    """Shared (per-core-identical) input arrays."""
    c = np.ascontiguousarray
    ew1 = _f32(kw["edge_w1"])                       # [128, 256]
    nw3 = _f32(kw["node_w3"])                       # [256, 64]
    M = nw3 @ ew1[:ODE]                             # [256, 256]
    bz_chan = ew1[:ODE].T @ _f32(kw["node_b3"]) + _f32(kw["edge_b1"])  # [256]
    # ew1[64:] is [64, 256]; lhsT columns for half h are ew1[64+p, 128h+c]
    wr = ew1[ODE:].reshape(64, 2, 128)              # [p, h, c]
    wrep = c(np.broadcast_to(wr[:, None, :, :], (64, NT_E, 2, 128)).astype(BF16NP))

    def halves2(b):   # [256] -> [128, 2]
        return c(_f32(b).reshape(2, 128).T)

    def pack_w(wm, cols):  # [256, cols] -> [128, 2, cols] with [p, k, m] = w[k*128+p, m]
        return c(_f32(wm).reshape(2, 128, cols).transpose(1, 0, 2).astype(BF16NP))

    return {
        "wrep": wrep,
        "nw1": c(_f32(kw["node_w1"]).astype(BF16NP)),
        "nw2": pack_w(kw["node_w2"], H),
        "ew2": pack_w(kw["edge_w2"], H),
        "ew3": pack_w(kw["edge_w3"], ODE),
        "Mt": pack_w(M, H),
        "nb1": halves2(kw["node_b1"]),
        "nb2": halves2(kw["node_b2"]),
        "bz": halves2(bz_chan),
        "eb2": halves2(kw["edge_b2"]),
        "eb3s": c(np.tile(_f32(kw["edge_b3"]), 2).reshape(128, 1)),
    }


def _x_transposed_per_core(x, cidx):
    xs = _f32(x).reshape(G, ODE, 2, NDATA)[cidx * GC:(cidx + 1) * GC]
    return np.ascontiguousarray(xs.transpose(1, 2, 0, 3).reshape(128, RC).astype(BF16NP))


def _attr2_per_core(edge_attr, cidx):
    at = np.empty((68, EC), dtype=BF16NP)
    at[0:64] = _f32(edge_attr)[cidx * EC:(cidx + 1) * EC].T.astype(BF16NP)
    gl = (np.arange(EC) // EPG) % 4                 # graph-in-tile index per edge col
    ind = (gl[None, :] == np.arange(4)[:, None])
    at[64:68] = ind.astype(BF16NP)
    return np.ascontiguousarray(at)


def _expand_out(outP):
    # outP [128, EC//2] bf16: [64q+ch, c] = out[2c+q, ch]
    o = _f32(outP).reshape(2, 64, EC // 2)
    return np.ascontiguousarray(o.transpose(2, 0, 1).reshape(EC, 64))


def _host_reference(kw, edge_attr, g_src, same):
    """Numpy fallback for non-structured inputs (never hit by the harness)."""
    x = _f32(kw["x"])
    xr = x.reshape(G, ODE, 2, NDATA)
    dp = xr.transpose(0, 3, 1, 2).reshape(G * NDATA, 2 * ODE)

    def mlp(h, w1, b1, w2, b2, w3, b3):
        h = np.maximum(h @ _f32(w1) + _f32(b1), 0)
        h = np.maximum(h @ _f32(w2) + _f32(b2), 0)
        return h @ _f32(w3) + _f32(b3)

    fe = mlp(dp, kw["node_w1"], kw["node_b1"], kw["node_w2"], kw["node_b2"],
             kw["node_w3"], kw["node_b3"]).reshape(G, NDATA, ODE).sum(1)
    attr_in = np.concatenate([fe[g_src], edge_attr], axis=1)
    new_attr = mlp(attr_in, kw["edge_w1"], kw["edge_b1"], kw["edge_w2"],
                   kw["edge_b2"], kw["edge_w3"], kw["edge_b3"])
    return np.where(same[:, None], new_attr, edge_attr)


def kernel(x, edge_attr, node_w1, node_b1, node_w2, node_b2, node_w3, node_b3,
           edge_w1, edge_b1, edge_w2, edge_b2, edge_w3, edge_b3,
           edge_index, batch):
    global last_results
    kw = dict(x=x, node_w1=node_w1, node_b1=node_b1, node_w2=node_w2,
              node_b2=node_b2, node_w3=node_w3, node_b3=node_b3,
              edge_w1=edge_w1, edge_b1=edge_b1, edge_w2=edge_w2,
              edge_b2=edge_b2, edge_w3=edge_w3, edge_b3=edge_b3)
    trace = os.environ.get("KERNEL_TRACE", "") == "1"
    if trace:
        _install_trace_shim()

    edge_attr = _f32(edge_attr)
    ei = np.asarray(edge_index)
    bt = np.asarray(batch)
    g_src = bt[ei[0]]
    same = (g_src == bt[ei[1]])
    structured = bool((g_src == np.repeat(np.arange(G), EPG)).all())
    if not structured:
        return _host_reference(kw, edge_attr, g_src, same)

    shared = _host_arrays(kw)
    in_maps = []
    for cidx in range(NCORES):
        m = dict(shared)
        m["xT"] = _x_transposed_per_core(x, cidx)
        m["attrT2"] = _attr2_per_core(edge_attr, cidx)
        in_maps.append(m)

    nc = _get_program()
    res = run_bass_kernel_spmd(nc, in_maps, core_ids=list(range(NCORES)),
                               trace=trace, trace_cores=[0] if trace else None)
    last_results = res
    out = np.empty((E, EA), dtype=np.float32)
    for cidx in range(NCORES):
        out[cidx * EC:(cidx + 1) * EC] = _expand_out(res.results[cidx]["outP"])
    if not same.all():
        out = np.where(same[:, None], out, edge_attr)
    return out
